# revision 1
# baseline (speedup 1.0000x reference)
"""Trainium2 Bass kernel for nn_DepthDCOp (per-pixel depthwise dynamic conv).

out[n,c,h,w] = sum_{i,j in 0..2} kernel[n,0,i*3+j,h,w] * xpad[n,c,h+i,w+j]
  (3x3 stencil, zero padding, per-pixel weights shared across channels)

Sharding: data-parallel over N — core i computes sample i (N == 8 == n_cores).

Per-core design (fp32):
  - x sample [256, 64*64] in SBUF as two c-tiles [128, PAD+4096+PAD] with a
    zeroed halo: every stencil tap is a plain free-dim offset read.
  - Kernel tap planes (w-edge columns of dw=±1 taps zeroed during host
    packing) are staged chunk-by-chunk into a partition-0 SBUF tile
    (partition_broadcast can only source partition 0 on hardware).
  - Per hw-chunk of 2048: GPSIMD partition_broadcast replicates each tap
    chunk across 128 partitions into SBUF; DVE (8 taps) / GPSIMD (1 tap)
    multiplies the shifted x window; PE accumulates all 9 products into a
    PSUM chunk with float32r identity matmuls (PSUM accumulation is fp32,
    so the adds cost no DVE/GPSIMD time); ACT drains PSUM to SBUF; DMA out.
"""

import os
import sys

import numpy as np

for _p in ("/opt/trn_rl_repo", "/root/.axon_site/_ro/trn_rl_repo"):
    if os.path.isdir(_p) and _p not in sys.path:
        sys.path.insert(0, _p)

import concourse.bass as bass  # noqa: E402
import concourse.bacc as bacc  # noqa: E402
import concourse.mybir as mybir  # noqa: E402
import concourse.tile as tile  # noqa: E402
from concourse.bass_utils import run_bass_kernel_spmd  # noqa: E402

N, C, H, W = 8, 256, 64, 64
HW = H * W  # 4096
K = 3
T = K * K  # 9 taps
PAD = 68  # halo on each side of the flattened hw axis (>= 65)
F32 = mybir.dt.float32
F32R = mybir.dt.float32r

CH = 2048  # hw chunk (4 PSUM banks)
NCH = HW // CH
POOL_TAPS = (4,)  # taps multiplied on GPSIMD; the rest on DVE

_cached = {}


def _build_nc():
    # Bacc.finalize() runs the sync-wait legalization passes (event-sem
    # splitting, matmul-wait relocation) that raw Bass skips.
    nc = bacc.Bacc(trn_type="TRN2")
    x_d = nc.dram_tensor("x", [C, HW], F32, kind="ExternalInput")
    k_d = nc.dram_tensor("ker", [T, HW], F32, kind="ExternalInput")
    i_d = nc.dram_tensor("ident", [128, 128], F32R, kind="ExternalInput")
    o_d = nc.dram_tensor("out", [C, HW], F32, kind="ExternalOutput")

    with tile.TileContext(nc) as tc:
        with (
            tc.tile_pool(name="xp", bufs=1) as xp,
            tc.tile_pool(name="kp", bufs=1) as kp,
            tc.tile_pool(name="kbcp", bufs=1) as kbcp,
            tc.tile_pool(name="kslp", bufs=3) as kslp,
            tc.tile_pool(name="prodp", bufs=3) as prodp,
            tc.tile_pool(name="outp", bufs=3) as outp,
            tc.tile_pool(name="pso", bufs=1, space="PSUM") as pso,
        ):
            # Both c-tiles side by side in one tile: the 2 muls per tap
            # merge into one double-length DVE op (halves per-op overhead).
            xt = xp.tile([128, 2, PAD + HW + PAD], F32, name="xt")
            ident = kp.tile([128, 128], F32R, name="ident")

            nc.vector.memset(xt[:, :, 0:PAD], 0.0)
            nc.vector.memset(xt[:, :, PAD + HW : PAD + HW + PAD], 0.0)
            nc.sync.dma_start(
                xt[:, :, PAD : PAD + HW],
                x_d.rearrange("(a p) w -> p a w", p=128)[:, :, :],
            )
            nc.sync.dma_start(ident[:, :], i_d[:, :])

            for ch in range(NCH):
                hw0 = ch * CH
                # Broadcast all 9 tap chunks across partitions into SBUF.
                kbc = []
                for t in range(T):
                    ksl = kslp.tile([1, CH], F32, tag="ksl", name=f"ks{ch}_{t}")
                    nc.sync.dma_start(ksl[:, :], k_d[t : t + 1, hw0 : hw0 + CH])
                    kb = kbcp.tile(
                        [128, 1, CH], F32, tag=f"kbc{t}", name=f"kb{ch}_{t}"
                    )
                    nc.gpsimd.partition_broadcast(kb[:, :, :], ksl[0:1, :])
                    kbc.append(kb)
                # One double-length mul per tap covers both c-tiles; both
                # per-ct PSUM accumulators live for the whole chunk so each
                # prod is consumed immediately by its two id-adds.
                po = [
                    pso.tile([128, CH], F32, tag=f"po{c}", name=f"po{c}_{ch}")
                    for c in range(2)
                ]
                for t in range(T):
                    i, j = t // K, t % K
                    off = PAD + (i - 1) * W + (j - 1) + hw0
                    xwin = xt[:, :, off : off + CH]
                    # Written as float32r so the PE may consume it
                    # (f32r matmuls stream at full rate).
                    prod = prodp.tile(
                        [128, 2, CH], F32R, tag="prod", name=f"pr{ch}_{t}"
                    )
                    eng = nc.gpsimd if t in POOL_TAPS else nc.vector
                    eng.tensor_mul(
                        prod[:, :, :], xwin, kbc[t][:, :, :].to_broadcast((128, 2, CH))
                    )
                    # fp32 PSUM accumulation via identity matmul (f32r
                    # streams at full PE rate) — no adds on DVE/GPSIMD.
                    for ct in range(2):
                        for b in range(CH // 512):
                            nc.tensor.matmul(
                                po[ct][:, b * 512 : (b + 1) * 512],
                                ident[:, :],
                                prod[:, ct, b * 512 : (b + 1) * 512],
                                start=(t == 0),
                                stop=(t == T - 1),
                            )
                for ct in range(2):
                    ot = outp.tile([128, CH], F32, tag="ot", name=f"ot{ct}_{ch}")
                    nc.scalar.copy(ot[:, :], po[ct][:, :])
                    nc.sync.dma_start(
                        o_d[ct * 128 : (ct + 1) * 128, hw0 : hw0 + CH], ot[:, :]
                    )

    nc.finalize()
    return nc


def get_nc():
    if "nc" not in _cached:
        _cached["nc"] = _build_nc()
    return _cached["nc"]


def _pack_ker(ker_n):
    """[1, 9, H, W] f32 -> [T, HW] with w-edge columns of dw=±1 taps
    zeroed (kills the w-wraparound reads on device)."""
    k = np.array(ker_n.reshape(T, H, W), dtype=np.float32)
    for t in range(T):
        j = t % K
        if j == 0:
            k[t, :, 0] = 0.0
        elif j == K - 1:
            k[t, :, W - 1] = 0.0
    return k.reshape(T, HW)


_IDENT = np.eye(128, dtype=np.float32)


def kernel(x, kernel, kernel_size=3, dilation=1, **_):
    x = np.ascontiguousarray(np.asarray(x), dtype=np.float32)
    ker = np.ascontiguousarray(np.asarray(kernel), dtype=np.float32)
    assert x.shape == (N, C, H, W), x.shape
    assert ker.shape == (N, 1, T, H, W), ker.shape

    nc = get_nc()
    in_maps = [
        {"x": x[n].reshape(C, HW), "ker": _pack_ker(ker[n]), "ident": _IDENT}
        for n in range(N)
    ]
    res = run_bass_kernel_spmd(
        nc,
        in_maps,
        list(range(N)),
        trace=bool(int(os.environ.get("DDC_TRACE", "0"))),
    )
    _cached["last_results"] = res
    out = np.stack([res.results[n]["out"].reshape(C, H, W) for n in range(N)])
    return out



# revision 3
# speedup vs baseline: 3.9474x; 3.9474x over previous
"""Trainium2 Bass kernel for nn_DepthDCOp (per-pixel depthwise dynamic conv).

out[n,c,h,w] = sum_{i,j in 0..2} kernel[n,0,i*3+j,h,w] * xpad[n,c,h+i,w+j]
  (3x3 stencil, zero padding, per-pixel weights shared across channels)

Sharding: data-parallel over N — core i computes sample i (N == 8 == n_cores).

Per-core design (bf16 in/out, fp32 PSUM accumulate):
  The stencil is recast as banded matmuls over the flattened hw axis.  For
  output pixels g = 128a+p (tile a), out^T[g, c] = sum_t k_t[g] *
  x^T[g + d_t, c] with tap offsets d_t in {-65..65}.  The host packs the
  per-pixel weights into band matrices B[a, b][q, p] = k_t[128a+p] at
  q = p + d_t - 128(b-1) (w-edge taps zeroed, h-edges fall outside the
  band), so each output tile is just

      out^T_a = sum_{b=0..2} B[a,b]^T @ x^T_{a+b-1}

  i.e. three 128-contraction matmuls accumulating in PSUM.  The PE does
  the shift+multiply+9-tap-reduce in one pass; ACT/DVE alternate on the
  PSUM->SBUF drains; DMA (x^T in, bands in, out^T out, all bf16) is the
  roofline.  x/out transposes happen on the host.
"""

import os
import sys

import numpy as np
import ml_dtypes

for _p in ("/opt/trn_rl_repo", "/root/.axon_site/_ro/trn_rl_repo"):
    if os.path.isdir(_p) and _p not in sys.path:
        sys.path.insert(0, _p)

import concourse.bass as bass  # noqa: E402
import concourse.bacc as bacc  # noqa: E402
import concourse.mybir as mybir  # noqa: E402
import concourse.tile as tile  # noqa: E402
from concourse.bass_utils import run_bass_kernel_spmd  # noqa: E402

N, C, H, W = 8, 256, 64, 64
HW = H * W  # 4096
K = 3
T = K * K  # 9 taps
BF16 = mybir.dt.bfloat16
F32 = mybir.dt.float32

P = 128           # pixels per tile (partition dim of out^T tiles)
NT = HW // P      # 32 hw tiles
XC = 4            # tiles per DMA chunk
NCK = NT // XC    # 8 chunks

_cached = {}


def _build_nc():
    nc = bacc.Bacc(trn_type="TRN2")
    xT_d = nc.dram_tensor("xT", [HW, C], BF16, kind="ExternalInput")
    bd_d = nc.dram_tensor("band", [P, NT * 3 * P], BF16, kind="ExternalInput")
    oT_d = nc.dram_tensor("outT", [HW, C], BF16, kind="ExternalOutput")

    with tile.TileContext(nc) as tc:
        with (
            tc.tile_pool(name="xp", bufs=1) as xp,
            tc.tile_pool(name="bp", bufs=1) as bp,
            tc.tile_pool(name="op", bufs=3) as op,
            tc.tile_pool(name="pso", bufs=8, space="PSUM") as pso,
        ):
            # Per-chunk SBUF tiles (separate tiles => DMA/compute overlap at
            # chunk granularity in the tile dependency tracker).
            xts = [
                xp.tile([P, XC, C], BF16, name=f"xt{s}") for s in range(NCK)
            ]
            bds = [
                bp.tile([P, XC, 3, P], BF16, name=f"bd{s}") for s in range(NCK)
            ]
            xr = xT_d.rearrange("(a p) c -> p a c", p=P)
            for s in range(NCK):
                nc.sync.dma_start(xts[s][:, :, :], xr[:, s * XC : (s + 1) * XC, :])
                nc.sync.dma_start(
                    bds[s][:, :, :, :],
                    bd_d.rearrange("q (a b p) -> q a b p", b=3, p=P)[
                        :, s * XC : (s + 1) * XC, :, :
                    ],
                )

            orr = oT_d.rearrange("(a p) c -> p a c", p=P)
            for s in range(NCK):
                ot = op.tile([P, XC, C], BF16, tag="ot", name=f"ot{s}")
                for i in range(XC):
                    a = s * XC + i
                    po = pso.tile([P, C], F32, tag="po", name=f"po{a}")
                    bs = [b for b in range(3) if 0 <= a + b - 1 < NT]
                    for b in bs:
                        m = a + b - 1
                        nc.tensor.matmul(
                            po[:, :],
                            bds[s][:, i, b, :],
                            xts[m // XC][:, m % XC, :],
                            start=(b == bs[0]),
                            stop=(b == bs[-1]),
                        )
                    # Alternate drain engine so neither ACT nor DVE is the
                    # bottleneck.
                    if a % 2 == 0:
                        nc.scalar.copy(ot[:, i, :], po[:, :])
                    else:
                        nc.vector.tensor_copy(ot[:, i, :], po[:, :])
                nc.sync.dma_start(
                    orr[:, s * XC : (s + 1) * XC, :], ot[:, :, :]
                )

    nc.finalize()
    return nc


def get_nc():
    if "nc" not in _cached:
        _cached["nc"] = _build_nc()
    return _cached["nc"]


# Tap offsets in flattened hw space (i-1)*W + (j-1), torch Unfold order.
_DELTAS = [(t // K - 1) * W + (t % K - 1) for t in range(T)]


def _pack_band(ker_n):
    """[1, T, H, W] f32 -> [P, NT*3*P] bf16 band matrices.

    B[a, b][q, p] = k_t[128a+p] where 128(a+b-1)+q == 128a+p+d_t, with
    w-edge columns of j==0/j==2 taps zeroed (kills w wraparound) and
    h-out-of-range taps dropped (zero padding).
    Returned in DRAM layout [q, (a b p)].
    """
    k = np.array(ker_n.reshape(T, H, W), dtype=np.float32)
    for t in range(T):
        j = t % K
        if j == 0:
            k[t, :, 0] = 0.0
        elif j == K - 1:
            k[t, :, W - 1] = 0.0
    kf = k.reshape(T, HW)

    band = np.zeros((NT, 3, P, P), dtype=np.float32)  # [a, b, q, p]
    g = np.arange(HW)
    a, p = g >> 7, g & 127
    for t in range(T):
        gs = g + _DELTAS[t]
        v = (gs >= 0) & (gs < HW)
        q, b = gs & 127, (gs >> 7) - a + 1
        band[a[v], b[v], q[v], p[v]] = kf[t, v]
    # -> [q, a, b, p]
    return np.ascontiguousarray(band.transpose(2, 0, 1, 3)).reshape(
        P, NT * 3 * P
    ).astype(ml_dtypes.bfloat16)


def kernel(x, kernel, kernel_size=3, dilation=1, **_):
    x = np.asarray(x, dtype=np.float32)
    ker = np.asarray(kernel, dtype=np.float32)
    assert x.shape == (N, C, H, W), x.shape
    assert ker.shape == (N, 1, T, H, W), ker.shape

    nc = get_nc()
    in_maps = [
        {
            "xT": np.ascontiguousarray(
                x[n].reshape(C, HW).T.astype(ml_dtypes.bfloat16)
            ),
            "band": _pack_band(ker[n]),
        }
        for n in range(N)
    ]
    res = run_bass_kernel_spmd(
        nc,
        in_maps,
        list(range(N)),
        trace=bool(int(os.environ.get("DDC_TRACE", "0"))),
    )
    _cached["last_results"] = res
    out = np.stack(
        [
            np.asarray(res.results[n]["outT"], dtype=np.float32).T.reshape(
                C, H, W
            )
            for n in range(N)
        ]
    )
    return out
